# revision 15
# baseline (speedup 1.0000x reference)
"""Trainium2 Bass kernel for pairwise diagonal-Gaussian KL energies.

energies[b, i] = 0.5 * sum_d [ log(d_id) + (1 + (x_bd - mu_id)^2) / d_id - 1 ]
with d = clip(diag, 1e-6),  x: (4096, 128), mean/diag: (8192, 128).

Sharding: tensor-parallel over codebook rows (n_in) across 8 cores.
Each core gets the full x (host-transposed to [dim, batch], cast bf16) and a
1024-row shard of mean/diag (host-transposed, cast bf16), and produces the
(1024, batch) row slab of the transposed output in fp16; the host
concatenates along n_in, transposes back to (batch, n_in), casts f32.

Design ([n_in(partition), batch(free)] output orientation):
  - PSUM tiles are [128 n_in, 512 batch]: stationary = codebook tile
    (invb/minvb [dim, 128] bf16), moving = batch data (xxb/xb [dim, 512]
    bf16).  Per 128-row i-tile: 8 matmuls with invb_it (start) + 8 with
    minvb_it (accumulate) -> 2 stationary swaps per i-tile, PE streams
    1 col/cycle bf16: 65.5k cycles @2.4GHz = 27.3 us/pass (bottleneck).
  - The per-codebook constant is a PER-PARTITION [128,1] vector here, so
    evacuation fuses it for free: ACT Identity(ps + bias) / DVE
    tensor_scalar_add alternating per 512-chunk; no extra PE work.
  - fp16 output: rel-err budget is 2e-2, fp16 adds ~2e-4; output DMA
    halves vs f32 to ~25 us/core (f32 was DMA-bound at ~51 us).
  - inv carries the 0.5 quad scale: inv_half = recip_approx_fast(2*clip(d))
    so invb = bf16(0.5/d), xxb = plain x^2 (fast TensorTensor / ACT
    Square), minvb = -2*mu*inv_half = -mu/d, and the ln(2) offset
    constant-folds into the colsum bias (-64*(1+ln2)).
  - prep: diag/mean ship as bf16 (halved DMA), both ACT tables warm at
    t=0, codebook chain in 512-col halves spread over DVE/ACT/Pool,
    colsum via 8 tiny N=1 matmuls (stationary = s2 i-tile), one DVE
    tensor_scalar_add finalizes cvt [128,8].
"""

import numpy as np

N_IN, DIM, BATCH = 8192, 128, 4096
N_CORES = 8
SHARD = N_IN // N_CORES  # 1024 codebook rows per core
PD_THR = 1e-6
IT = SHARD // 128  # 8 i-tiles of 128 codebook rows
BC = BATCH // 512  # 8 batch chunks of 512 per i-tile
CVT_BIAS = -float(DIM // 2) * (1.0 + float(np.log(2.0)))  # -64*(1+ln2)

_BUILD_CACHE = {}


def build(
    repeat=1,
    psum_bufs=8,
    out_bufs=3,
    out_dma_engines=("sync",),
    evac_pattern="AVAVAVAV",  # per batch-chunk engine: A=ACT, V=DVE, P=Pool
    out_split=1,  # output DMAs per i-tile (1 = one [128,4096], 2 = two halves)
    skip_mm=False,
    skip_evac=False,
    skip_out_dma=False,
    out_dtype="f16",
):
    """Build + compile the single-core SPMD program. Cached per config."""
    key = (
        repeat, psum_bufs, out_bufs, out_dma_engines, evac_pattern, out_split,
        skip_mm, skip_evac, skip_out_dma, out_dtype,
    )
    if key in _BUILD_CACHE:
        return _BUILD_CACHE[key]

    import contextlib

    import concourse.bass as bass
    import concourse.bacc as bacc
    import concourse.tile as tile
    import concourse.mybir as mybir

    f32 = mybir.dt.float32
    bf16 = mybir.dt.bfloat16
    f16 = mybir.dt.float16
    AF = mybir.ActivationFunctionType
    ALU = mybir.AluOpType

    nc = bacc.Bacc("TRN2", target_bir_lowering=False, debug=False)

    odt = {"f16": f16, "f32": f32, "bf16": bf16}[out_dtype]
    xb_d = nc.dram_tensor("xb", [DIM, BATCH], bf16, kind="ExternalInput")
    mt_d = nc.dram_tensor("meant", [DIM, SHARD], bf16, kind="ExternalInput")
    dg_d = nc.dram_tensor("diagt", [DIM, SHARD], bf16, kind="ExternalInput")
    out_d = nc.dram_tensor("out", [SHARD, BATCH], odt, kind="ExternalOutput")
    out_ap = out_d.ap()

    with tile.TileContext(nc) as tc:
        with (
            tc.tile_pool(name="persist", bufs=1) as pp,
            tc.tile_pool(name="prep", bufs=1) as prep,
        ):
            zb = pp.tile([DIM, 1], f32)
            nc.gpsimd.memset(zb[:], 0.0)
            ones_col = pp.tile([DIM, 1], f32)
            nc.gpsimd.memset(ones_col[:], 1.0)

            # warm BOTH ACT tables (Square -> set0, Ln -> natural_log)
            # before anything else queues on the ACT sequencer, so both
            # ATLs overlap the input-DMA window
            warm = prep.tile([DIM, 1], f32)
            nc.scalar.activation(warm[:], zb[:], AF.Square, bias=zb[:])
            nc.scalar.activation(warm[:], zb[:], AF.Ln, bias=zb[:])

            # ---- input loads, all on the (otherwise idle) sync ring; the
            # transfers serialize on the DMA queue anyway, so order =
            # dependency order: diag, mean, then x (half1 first: ACT's
            # Square chunks read it).
            dg = prep.tile([DIM, SHARD], bf16)
            mt = prep.tile([DIM, SHARD], bf16)
            xb = pp.tile([DIM, BATCH], bf16)
            xhalf = BATCH // 2
            nc.sync.dma_start(dg[:], dg_d.ap())
            nc.sync.dma_start(mt[:], mt_d.ap())
            nc.sync.dma_start(xb[:, xhalf:], xb_d.ap()[:, xhalf:])
            nc.sync.dma_start(xb[:, :xhalf], xb_d.ap()[:, :xhalf])

            # ---- codebook prep in 512-col halves; xxb in 1024-col chunks
            dc = prep.tile([DIM, SHARD], f32)   # 2*clip(diag)
            lg = prep.tile([DIM, SHARD], f32)   # ln(2*clip(diag))
            lgh = prep.tile([DIM, SHARD], f32)  # 0.5*ln(2*clip(diag))
            inv = prep.tile([DIM, SHARD], f32)  # 0.5/clip(diag)
            invb = pp.tile([DIM, SHARD], bf16)
            minvb = pp.tile([DIM, SHARD], bf16)
            m2 = prep.tile([DIM, SHARD], f32)
            t2 = prep.tile([DIM, SHARD], f32)
            s2 = prep.tile([DIM, SHARD], f32)
            xxb = pp.tile([DIM, BATCH], bf16)
            cvt = pp.tile([DIM, IT], f32)  # per-codebook constant, [128, 8]
            with tc.tile_pool(
                name="psum_prep", bufs=1, space=bass.MemorySpace.PSUM
            ) as psp:
                cps = psp.tile([DIM, IT], f32)
                XQ = BATCH // 4  # 1024-col x^2 chunks
                H = [slice(0, 512), slice(512, 1024)]
                XC = [slice(i * XQ, (i + 1) * XQ) for i in range(4)]
                # Per-engine FIFOs, emitted in intended execution order.
                # DVE: dc -> recip -> invb -> minvb -> s2, then one x^2 chunk
                for sl in H:
                    nc.vector.tensor_scalar(
                        dc[:, sl], dg[:, sl], PD_THR, 2.0, ALU.max, ALU.mult
                    )
                for sl in H:
                    nc.vector.reciprocal_approx_fast(inv[:, sl], dc[:, sl])
                # ACT: ln(2*clip(diag)), then squares (x half1 lands first)
                for sl in H:
                    nc.scalar.activation(lg[:, sl], dc[:, sl], AF.Ln, bias=zb[:])
                # Pool: m2 + invb cast (TT/copy only — Pool codegen
                # rejects scalar_tensor_tensor)
                for sl in H:
                    nc.gpsimd.tensor_mul(m2[:, sl], mt[:, sl], mt[:, sl])
                for sl in H:
                    nc.gpsimd.tensor_copy(invb[:, sl], inv[:, sl])
                for sl in H:
                    nc.vector.scalar_tensor_tensor(
                        minvb[:, sl], mt[:, sl], -2.0, inv[:, sl],
                        ALU.mult, ALU.mult,
                    )
                for sl in H:
                    nc.vector.scalar_tensor_tensor(
                        t2[:, sl], m2[:, sl], 1.0, inv[:, sl], ALU.add, ALU.mult
                    )
                for cs in (XC[2], XC[3], XC[0]):
                    nc.scalar.activation(
                        xxb[:, cs], xb[:, cs], AF.Square, bias=zb[:]
                    )
                # s2 = 0.5*lg + t2; colsum via N=1 matmul per i-tile
                for h, sl in enumerate(H):
                    nc.vector.scalar_tensor_tensor(
                        s2[:, sl], lg[:, sl], 0.5, t2[:, sl], ALU.mult, ALU.add
                    )
                    for it in range(h * (IT // 2), (h + 1) * (IT // 2)):
                        isl = slice(it * 128, (it + 1) * 128)
                        nc.tensor.matmul(
                            cps[:, it : it + 1], s2[:, isl], ones_col[:]
                        )
                # last x^2 chunk split DVE/Pool so the DVE tail stays short
                c1a = slice(XC[1].start, XC[1].start + 512)
                c1b = slice(XC[1].start + 512, XC[1].stop)
                nc.gpsimd.tensor_mul(xxb[:, c1b], xb[:, c1b], xb[:, c1b])
                nc.vector.tensor_mul(xxb[:, c1a], xb[:, c1a], xb[:, c1a])
                # cvt = cps - 64*(1+ln2), one op over [128, 8]
                nc.vector.tensor_scalar_add(cvt[:], cps[:], CVT_BIAS)

            # ---- main loop ----
            with (
                tc.tile_pool(
                    name="psum", bufs=psum_bufs, space=bass.MemorySpace.PSUM
                ) as psm,
                tc.tile_pool(name="outs", bufs=out_bufs) as osp,
            ):
                loop_cm = (
                    tc.For_i(0, repeat, 1) if repeat > 1 else contextlib.nullcontext()
                )
                with loop_cm:
                    for it in range(IT):
                        isl = slice(it * 128, (it + 1) * 128)
                        ob = osp.tile([128, BATCH], odt)
                        pss = []

                        def evac(b):
                            bs = slice(b * 512, (b + 1) * 512)
                            e = evac_pattern[b % len(evac_pattern)]
                            src = pss[b][:] if not skip_mm else xxb[:, bs]
                            if e == "A":
                                nc.scalar.activation(
                                    ob[:, bs], src, AF.Identity,
                                    bias=cvt[:, it : it + 1],
                                )
                            elif e == "V":
                                nc.vector.tensor_scalar_add(
                                    ob[:, bs], src, cvt[:, it : it + 1]
                                )
                            else:  # Pool: TT with free-dim-broadcast bias
                                nc.gpsimd.tensor_add(
                                    ob[:, bs], src,
                                    cvt[:, it : it + 1].broadcast_to([128, 512]),
                                )

                        def out_dma(part, nparts):
                            w = BATCH // nparts
                            cs = slice(part * w, (part + 1) * w)
                            eng = getattr(
                                nc,
                                out_dma_engines[
                                    (it * nparts + part) % len(out_dma_engines)
                                ],
                            )
                            eng.dma_start(out_ap[isl, cs], ob[:, cs])

                        if not skip_mm:
                            for b in range(BC):
                                bs = slice(b * 512, (b + 1) * 512)
                                ps = psm.tile([128, 512], f32)
                                nc.tensor.matmul(
                                    ps[:], invb[:, isl], xxb[:, bs],
                                    start=True, stop=False,
                                )
                                pss.append(ps)
                            for b in range(BC):
                                bs = slice(b * 512, (b + 1) * 512)
                                nc.tensor.matmul(
                                    pss[b][:], minvb[:, isl], xb[:, bs],
                                    start=False, stop=True,
                                )
                                if not skip_evac:
                                    evac(b)
                                    if (
                                        not skip_out_dma
                                        and out_split > 1
                                        and (b + 1) % (BC // out_split) == 0
                                    ):
                                        out_dma(
                                            (b + 1) // (BC // out_split) - 1,
                                            out_split,
                                        )
                        elif not skip_evac:
                            for b in range(BC):
                                evac(b)
                        if not (skip_out_dma or skip_evac) and out_split == 1:
                            out_dma(0, 1)

    nc.compile()
    _BUILD_CACHE[key] = nc
    return nc


def make_in_maps(x, mean, diag):
    import ml_dtypes

    bf = ml_dtypes.bfloat16
    xb = np.ascontiguousarray(np.asarray(x).T.astype(bf))
    in_maps = []
    for c in range(N_CORES):
        sl = slice(c * SHARD, (c + 1) * SHARD)
        in_maps.append(
            {
                "xb": xb,
                "meant": np.ascontiguousarray(np.asarray(mean)[sl].T.astype(bf)),
                "diagt": np.ascontiguousarray(np.asarray(diag)[sl].T.astype(bf)),
            }
        )
    return in_maps


def kernel(x, mean, diag):
    from concourse.bass_utils import run_bass_kernel_spmd

    nc = build(repeat=1)
    in_maps = make_in_maps(x, mean, diag)
    try:
        res = run_bass_kernel_spmd(nc, in_maps, list(range(N_CORES)))
    except Exception:
        # rare transient device error; one retry
        res = run_bass_kernel_spmd(nc, in_maps, list(range(N_CORES)))
    # per-core out is (SHARD, BATCH) = energies.T slab; stack along n_in,
    # transpose back to (batch, n_in), cast to f32
    full_t = np.concatenate(
        [np.asarray(res.results[c]["out"]) for c in range(N_CORES)], axis=0
    )
    return np.ascontiguousarray(full_t.T).astype(np.float32)


# revision 16
# speedup vs baseline: 1.0718x; 1.0718x over previous
"""Trainium2 Bass kernel for pairwise diagonal-Gaussian KL energies.

energies[b, i] = 0.5 * sum_d [ log(d_id) + (1 + (x_bd - mu_id)^2) / d_id - 1 ]
with d = clip(diag, 1e-6),  x: (4096, 128), mean/diag: (8192, 128).

Sharding: tensor-parallel over codebook rows (n_in) across 8 cores.
Each core gets the full x (host-transposed to [dim, batch], cast bf16) and a
1024-row shard of mean/diag (host-transposed, cast bf16), and produces the
(1024, batch) row slab of the transposed output in fp16; the host
concatenates along n_in, transposes back to (batch, n_in), casts f32.

Design ([n_in(partition), batch(free)] output orientation):
  - PSUM tiles are [128 n_in, 512 batch]: stationary = codebook tile
    (invb/minvb [dim, 128] bf16), moving = batch data (xxb/xb [dim, 512]
    bf16).  Per 128-row i-tile: 8 matmuls with invb_it (start) + 8 with
    minvb_it (accumulate) -> 2 stationary swaps per i-tile.
  - The per-codebook constant is a PER-PARTITION [128,1] vector here, so
    evacuation fuses it for free: ACT Identity(ps + bias) / DVE
    tensor_scalar_add alternating per 512-chunk; no extra PE work.
  - fp16 output: rel-err budget is 2e-2, fp16 adds ~2e-4; output DMA
    halves vs f32 to ~25 us/core (f32 was DMA-bound at ~51 us).
  - inv carries the 0.5 quad scale: inv_half = recip_approx_fast(2*clip(d))
    so invb = bf16(0.5/d), xxb = plain x^2 (fast TensorTensor / ACT
    Square), minvb = -2*mu*inv_half = -mu/d, and the ln(2) offset
    constant-folds into the colsum bias (-64*(1+ln2)).
  - prep: diag/mean ship as bf16 (halved DMA), both ACT tables warm at
    t=0, codebook chain in 512-col halves spread over DVE/ACT/Pool,
    colsum via 8 tiny N=1 matmuls (stationary = s2 i-tile), one DVE
    tensor_scalar_add finalizes cvt [128,8].

Measured on 8x trn2 NC (repeat-slope): pure-PE 32.7 us/pass (the
streaming model says 27.3; walrus runs with ldw-opt off and each matmul
pays ~42 ns extra), +3 us with evacuation, +4.3 us with the output DMA
(SBUF-port contention) -> ~40 us/pass steady state.  Evac mixes
(all-ACT 44, all-DVE 50, 4A/4V 40), psum_bufs {4,6,8}, out-DMA
splitting/dual-ring: all within noise of 4A/4V+psum8+single-DMA.
Prep (cost model, outside the repeat loop): 10.3 us.  Max elementwise
rel err vs the f32 jax reference: 3.0e-3 (bf16 GEMM operands + bf16
mean/diag inputs + fp16 output; gate is 2e-2).
"""

import numpy as np

N_IN, DIM, BATCH = 8192, 128, 4096
N_CORES = 8
SHARD = N_IN // N_CORES  # 1024 codebook rows per core
PD_THR = 1e-6
IT = SHARD // 128  # 8 i-tiles of 128 codebook rows
BC = BATCH // 512  # 8 batch chunks of 512 per i-tile
CVT_BIAS = -float(DIM // 2) * (1.0 + float(np.log(2.0)))  # -64*(1+ln2)

_BUILD_CACHE = {}


def build(
    repeat=1,
    psum_bufs=8,
    out_bufs=3,
    out_dma_engines=("sync",),
    evac_pattern="AVAVAVAV",  # per batch-chunk engine: A=ACT, V=DVE, P=Pool
    out_split=1,  # output DMAs per i-tile (1 = one [128,4096], 2 = two halves)
    skip_mm=False,
    skip_evac=False,
    skip_out_dma=False,
    out_dtype="f16",
):
    """Build + compile the single-core SPMD program. Cached per config."""
    key = (
        repeat, psum_bufs, out_bufs, out_dma_engines, evac_pattern, out_split,
        skip_mm, skip_evac, skip_out_dma, out_dtype,
    )
    if key in _BUILD_CACHE:
        return _BUILD_CACHE[key]

    import contextlib

    import concourse.bass as bass
    import concourse.bacc as bacc
    import concourse.tile as tile
    import concourse.mybir as mybir

    f32 = mybir.dt.float32
    bf16 = mybir.dt.bfloat16
    f16 = mybir.dt.float16
    AF = mybir.ActivationFunctionType
    ALU = mybir.AluOpType

    nc = bacc.Bacc("TRN2", target_bir_lowering=False, debug=False)

    odt = {"f16": f16, "f32": f32, "bf16": bf16}[out_dtype]
    xb_d = nc.dram_tensor("xb", [DIM, BATCH], bf16, kind="ExternalInput")
    mt_d = nc.dram_tensor("meant", [DIM, SHARD], bf16, kind="ExternalInput")
    dg_d = nc.dram_tensor("diagt", [DIM, SHARD], bf16, kind="ExternalInput")
    out_d = nc.dram_tensor("out", [SHARD, BATCH], odt, kind="ExternalOutput")
    out_ap = out_d.ap()

    with tile.TileContext(nc) as tc:
        with (
            tc.tile_pool(name="persist", bufs=1) as pp,
            tc.tile_pool(name="prep", bufs=1) as prep,
        ):
            zb = pp.tile([DIM, 1], f32)
            nc.gpsimd.memset(zb[:], 0.0)
            ones_col = pp.tile([DIM, 1], f32)
            nc.gpsimd.memset(ones_col[:], 1.0)

            # warm BOTH ACT tables (Square -> set0, Ln -> natural_log)
            # before anything else queues on the ACT sequencer, so both
            # ATLs overlap the input-DMA window
            warm = prep.tile([DIM, 1], f32)
            nc.scalar.activation(warm[:], zb[:], AF.Square, bias=zb[:])
            nc.scalar.activation(warm[:], zb[:], AF.Ln, bias=zb[:])

            # ---- input loads, all on the (otherwise idle) sync ring; the
            # transfers serialize on the DMA queue anyway, so order =
            # dependency order: diag, mean, then x (half1 first: ACT's
            # Square chunks read it).
            dg = prep.tile([DIM, SHARD], bf16)
            mt = prep.tile([DIM, SHARD], bf16)
            xb = pp.tile([DIM, BATCH], bf16)
            xhalf = BATCH // 2
            nc.sync.dma_start(dg[:], dg_d.ap())
            nc.sync.dma_start(mt[:], mt_d.ap())
            nc.sync.dma_start(xb[:, xhalf:], xb_d.ap()[:, xhalf:])
            nc.sync.dma_start(xb[:, :xhalf], xb_d.ap()[:, :xhalf])

            # ---- codebook prep in 512-col halves; xxb in 1024-col chunks
            dc = prep.tile([DIM, SHARD], f32)   # 2*clip(diag)
            lg = prep.tile([DIM, SHARD], f32)   # ln(2*clip(diag))
            lgh = prep.tile([DIM, SHARD], f32)  # 0.5*ln(2*clip(diag))
            inv = prep.tile([DIM, SHARD], f32)  # 0.5/clip(diag)
            invb = pp.tile([DIM, SHARD], bf16)
            minvb = pp.tile([DIM, SHARD], bf16)
            m2 = prep.tile([DIM, SHARD], f32)
            t2 = prep.tile([DIM, SHARD], f32)
            s2 = prep.tile([DIM, SHARD], f32)
            xxb = pp.tile([DIM, BATCH], bf16)
            cvt = pp.tile([DIM, IT], f32)  # per-codebook constant, [128, 8]
            with tc.tile_pool(
                name="psum_prep", bufs=1, space=bass.MemorySpace.PSUM
            ) as psp:
                cps = psp.tile([DIM, IT], f32)
                XQ = BATCH // 4  # 1024-col x^2 chunks
                H = [slice(0, 512), slice(512, 1024)]
                XC = [slice(i * XQ, (i + 1) * XQ) for i in range(4)]
                # Per-engine FIFOs, emitted in intended execution order.
                # DVE: dc -> recip -> invb -> minvb -> s2, then one x^2 chunk
                for sl in H:
                    nc.vector.tensor_scalar(
                        dc[:, sl], dg[:, sl], PD_THR, 2.0, ALU.max, ALU.mult
                    )
                for sl in H:
                    nc.vector.reciprocal_approx_fast(inv[:, sl], dc[:, sl])
                # ACT: ln(2*clip(diag)), then squares (x half1 lands first)
                for sl in H:
                    nc.scalar.activation(lg[:, sl], dc[:, sl], AF.Ln, bias=zb[:])
                # Pool: m2 + invb cast (TT/copy only — Pool codegen
                # rejects scalar_tensor_tensor)
                for sl in H:
                    nc.gpsimd.tensor_mul(m2[:, sl], mt[:, sl], mt[:, sl])
                for sl in H:
                    nc.gpsimd.tensor_copy(invb[:, sl], inv[:, sl])
                for sl in H:
                    nc.vector.scalar_tensor_tensor(
                        minvb[:, sl], mt[:, sl], -2.0, inv[:, sl],
                        ALU.mult, ALU.mult,
                    )
                for sl in H:
                    nc.vector.scalar_tensor_tensor(
                        t2[:, sl], m2[:, sl], 1.0, inv[:, sl], ALU.add, ALU.mult
                    )
                for cs in (XC[2], XC[3], XC[0]):
                    nc.scalar.activation(
                        xxb[:, cs], xb[:, cs], AF.Square, bias=zb[:]
                    )
                # s2 = 0.5*lg + t2; colsum via N=1 matmul per i-tile
                for h, sl in enumerate(H):
                    nc.vector.scalar_tensor_tensor(
                        s2[:, sl], lg[:, sl], 0.5, t2[:, sl], ALU.mult, ALU.add
                    )
                    for it in range(h * (IT // 2), (h + 1) * (IT // 2)):
                        isl = slice(it * 128, (it + 1) * 128)
                        nc.tensor.matmul(
                            cps[:, it : it + 1], s2[:, isl], ones_col[:]
                        )
                # last x^2 chunk split DVE/Pool so the DVE tail stays short
                c1a = slice(XC[1].start, XC[1].start + 512)
                c1b = slice(XC[1].start + 512, XC[1].stop)
                nc.gpsimd.tensor_mul(xxb[:, c1b], xb[:, c1b], xb[:, c1b])
                nc.vector.tensor_mul(xxb[:, c1a], xb[:, c1a], xb[:, c1a])
                # cvt = cps - 64*(1+ln2), one op over [128, 8]
                nc.vector.tensor_scalar_add(cvt[:], cps[:], CVT_BIAS)

            # ---- main loop ----
            with (
                tc.tile_pool(
                    name="psum", bufs=psum_bufs, space=bass.MemorySpace.PSUM
                ) as psm,
                tc.tile_pool(name="outs", bufs=out_bufs) as osp,
            ):
                loop_cm = (
                    tc.For_i(0, repeat, 1) if repeat > 1 else contextlib.nullcontext()
                )
                with loop_cm:
                    for it in range(IT):
                        isl = slice(it * 128, (it + 1) * 128)
                        ob = osp.tile([128, BATCH], odt)
                        pss = []

                        def evac(b):
                            bs = slice(b * 512, (b + 1) * 512)
                            e = evac_pattern[b % len(evac_pattern)]
                            src = pss[b][:] if not skip_mm else xxb[:, bs]
                            if e == "A":
                                nc.scalar.activation(
                                    ob[:, bs], src, AF.Identity,
                                    bias=cvt[:, it : it + 1],
                                )
                            elif e == "V":
                                nc.vector.tensor_scalar_add(
                                    ob[:, bs], src, cvt[:, it : it + 1]
                                )
                            else:  # Pool: TT with free-dim-broadcast bias
                                nc.gpsimd.tensor_add(
                                    ob[:, bs], src,
                                    cvt[:, it : it + 1].broadcast_to([128, 512]),
                                )

                        def out_dma(part, nparts):
                            w = BATCH // nparts
                            cs = slice(part * w, (part + 1) * w)
                            eng = getattr(
                                nc,
                                out_dma_engines[
                                    (it * nparts + part) % len(out_dma_engines)
                                ],
                            )
                            eng.dma_start(out_ap[isl, cs], ob[:, cs])

                        if not skip_mm:
                            for b in range(BC):
                                bs = slice(b * 512, (b + 1) * 512)
                                ps = psm.tile([128, 512], f32)
                                nc.tensor.matmul(
                                    ps[:], invb[:, isl], xxb[:, bs],
                                    start=True, stop=False,
                                )
                                pss.append(ps)
                            for b in range(BC):
                                bs = slice(b * 512, (b + 1) * 512)
                                nc.tensor.matmul(
                                    pss[b][:], minvb[:, isl], xb[:, bs],
                                    start=False, stop=True,
                                )
                                if not skip_evac:
                                    evac(b)
                                    if (
                                        not skip_out_dma
                                        and out_split > 1
                                        and (b + 1) % (BC // out_split) == 0
                                    ):
                                        out_dma(
                                            (b + 1) // (BC // out_split) - 1,
                                            out_split,
                                        )
                        elif not skip_evac:
                            for b in range(BC):
                                evac(b)
                        if not (skip_out_dma or skip_evac) and out_split == 1:
                            out_dma(0, 1)

    nc.compile()
    _BUILD_CACHE[key] = nc
    return nc


def make_in_maps(x, mean, diag):
    import ml_dtypes

    bf = ml_dtypes.bfloat16
    xb = np.ascontiguousarray(np.asarray(x).T.astype(bf))
    in_maps = []
    for c in range(N_CORES):
        sl = slice(c * SHARD, (c + 1) * SHARD)
        in_maps.append(
            {
                "xb": xb,
                "meant": np.ascontiguousarray(np.asarray(mean)[sl].T.astype(bf)),
                "diagt": np.ascontiguousarray(np.asarray(diag)[sl].T.astype(bf)),
            }
        )
    return in_maps


def kernel(x, mean, diag):
    from concourse.bass_utils import run_bass_kernel_spmd

    nc = build(repeat=1)
    in_maps = make_in_maps(x, mean, diag)
    try:
        res = run_bass_kernel_spmd(nc, in_maps, list(range(N_CORES)))
    except Exception:
        # rare transient device error; one retry
        res = run_bass_kernel_spmd(nc, in_maps, list(range(N_CORES)))
    # per-core out is (SHARD, BATCH) = energies.T slab; stack along n_in,
    # transpose back to (batch, n_in), cast to f32
    full_t = np.concatenate(
        [np.asarray(res.results[c]["out"]) for c in range(N_CORES)], axis=0
    )
    return np.ascontiguousarray(full_t.T).astype(np.float32)
